# revision 19
# baseline (speedup 1.0000x reference)
"""Causal self-attention (B=2, T=4096, C=768, H=12) on 8 TRN2 NeuronCores.

Sharding: core c -> batch c//4, heads 3*(c%4) .. 3*(c%4)+2.  Each core is
fully independent (no collectives): it computes qkv for its 3 heads from
x[b], runs causal flash attention, and produces the partial output
projection outT = (Y_heads @ W_proj[rows]).T of shape [C, T].  The host
sums the 4 per-batch partials, transposes, and adds the folded bias
(W_proj^T b_v + b_proj).

Per-core structure (all matmuls float32r, 1 cyc/row at N>=256):
  - qkT kept as 4 m-tiles A=[q0|q1], B=[k0|k1], C=[q2|k2], C'=[k2|q2]
    (host-packed weight layout) so every S^T matmul pair issues from PE
    row-groups {0,1} and {2,3} concurrently with NO duplicate copies:
    heads 0/1 pair across the partition halves of A/B; head 2 uses C/C'.
  - v in natural [T, 64] orientation with a ones column (softmax
    denominator rides in the PV matmul); the ones are injected by a K=1
    matmul against a constant pattern row, not DVE writes.
  - exp on ACT with no max subtraction (logits are bounded); diagonal
    k-tiles are column-compacted (only q >= k block computed) and masked
    with small [128,128/256] triangle multiplies on DVE.
  - softmax division: PSUM yt -> SBUF copy (frees the PSUM bank for the
    next group immediately), reciprocal_approx_fast, K=1 broadcast
    matmul, and one fused scalar_tensor_tensor multiply.
  - All side work (next group's QKV, previous group's output projection,
    epilogue broadcast/divide) flows through a deferred-thunk queue that
    is pumped between attention units, keeping the PE stream dense so
    the HAM clock gate stays at 8/8.
"""

import os
import sys

import numpy as np

for _p in ("/opt/trn_rl_repo", "/root/.axon_site/_ro/trn_rl_repo"):
    if os.path.isdir(_p) and _p not in sys.path:
        sys.path.insert(0, _p)

from collections import deque
from contextlib import ExitStack

import concourse.bacc as bacc
import concourse.bass as bass
import concourse.mybir as mybir
import concourse.tile as tile
from concourse.bass_utils import run_bass_kernel_spmd

F32 = mybir.dt.float32
F32R = mybir.dt.float32r
BF16 = mybir.dt.bfloat16
EXP = mybir.ActivationFunctionType.Exp
IS_GE = mybir.AluOpType.is_ge
MUL = mybir.AluOpType.mult

ECHO = int(os.environ.get("K_ECHO", "0"))

B, T_FULL, C = 2, 4096, 768
H, DH = 12, 64
HPC = 3                      # heads per core
NCORES = 8
P = 128
QG = 512                     # query-group span
KT = 128                     # key tile
NQK = 4 * P                  # 512 rows of qkT (A, B, C, C')
NVP = 256                    # padded v width: v0|1|v1|1|v2|1|zeros
SCALE = 1.0 / np.sqrt(DH)


def r32(ap):
    return ap.bitcast(F32R)


def build_nc(t=T_FULL):
    ng = t // QG             # query groups
    nc = bacc.Bacc(None, target_bir_lowering=False)
    xT = nc.declare_dram_parameter("xT", [P, 6 * t], BF16, isOutput=False)
    wqk = nc.declare_dram_parameter("wqk", [P, 6 * NQK], BF16, isOutput=False)
    bqk = nc.declare_dram_parameter("bqk", [P, 4], F32, isOutput=False)
    wvp = nc.declare_dram_parameter("wvp", [P, 6 * NVP], BF16, isOutput=False)
    wp = nc.declare_dram_parameter("wp", [HPC * DH, C], BF16, isOutput=False)
    outT = nc.declare_dram_parameter("outT", [C, t], F32, isOutput=True)

    with tile.TileContext(nc) as tc, ExitStack() as ctx:
        const = ctx.enter_context(tc.tile_pool(name="const", bufs=1))
        qkp = ctx.enter_context(tc.tile_pool(name="qk", bufs=1))
        vp = ctx.enter_context(tc.tile_pool(name="vn", bufs=1))
        xpool = ctx.enter_context(tc.tile_pool(name="xin", bufs=1))
        ppool = ctx.enter_context(tc.tile_pool(name="pp", bufs=4))
        ytsb = ctx.enter_context(tc.tile_pool(name="ytsb", bufs=2))
        ysbp = ctx.enter_context(tc.tile_pool(name="ysb", bufs=3))
        rrp = ctx.enter_context(tc.tile_pool(name="rr", bufs=3))
        osb = ctx.enter_context(tc.tile_pool(name="osb", bufs=2))
        spsum = ctx.enter_context(tc.tile_pool(name="sps", bufs=2, space="PSUM"))
        ytps = ctx.enter_context(tc.tile_pool(name="ytps", bufs=2, space="PSUM"))
        aux = ctx.enter_context(tc.tile_pool(name="aux", bufs=2, space="PSUM"))

        # ---- constants (packed single-DMA loads) -----------------------
        wqk_all = const.tile([P, 6 * NQK], BF16, tag="wqka", name="wqka")
        nc.sync.dma_start(wqk_all[:], wqk[:, :])
        wqk_sb = [wqk_all[:, k * NQK:(k + 1) * NQK] for k in range(6)]
        wvp_all = const.tile([P, 6 * NVP], BF16, tag="wvpa", name="wvpa")
        nc.sync.dma_start(wvp_all[:], wvp[:, :])
        wvp_sb = [wvp_all[:, k * NVP:(k + 1) * NVP] for k in range(6)]
        bq_all = const.tile([P, 4], F32, tag="bqa", name="bqa")
        nc.sync.dma_start(bq_all[:], bqk[:, :])
        b_sb = [bq_all[:, m:m + 1] for m in range(4)]
        wp0 = const.tile([P, C], BF16, tag="wp0", name="wp0")
        wp1 = const.tile([DH, C], BF16, tag="wp1", name="wp1")
        nc.sync.dma_start(wp0[:], wp[0:P, :])
        nc.sync.dma_start(wp1[:], wp[P:P + DH, :])

        ones1 = const.tile([1, DH], BF16, tag="ones1", name="ones1")
        ones1t = const.tile([1, P], BF16, tag="ones1t", name="ones1t")
        vpat = const.tile([1, NVP], BF16, tag="vpat", name="vpat")
        tri1 = const.tile([P, KT], BF16, tag="tri1", name="tri1")
        tri2 = const.tile([P, 2 * KT], BF16, tag="tri2", name="tri2")
        with tc.tile_pool(name="scratch", bufs=1) as scratch:
            onesRF = scratch.tile([1, P], F32, tag="onesRF", name="onesRF")
            nc.vector.memset(onesRF[:], 1.0)
            nc.vector.tensor_copy(ones1[:], onesRF[:, 0:DH])
            nc.vector.tensor_copy(ones1t[:], onesRF[:])
            vpF = scratch.tile([1, NVP], F32, tag="vpF", name="vpF")
            nc.vector.memset(vpF[:], 0.0)
            for h in range(HPC):
                nc.vector.memset(vpF[:, 65 * h + DH:65 * h + DH + 1], 1.0)
            nc.vector.tensor_copy(vpat[:], vpF[:])
            # tri1[k, q] = 1 iff q >= k ; tri2[k, q] = 1 iff q >= k + 128
            trF = scratch.tile([P, 2 * KT], F32, tag="trF", name="trF")
            nc.gpsimd.memset(trF[:], 1.0)
            nc.gpsimd.affine_select(
                out=trF[:, 0:KT], in_=trF[:, 0:KT], compare_op=IS_GE,
                fill=0.0, base=0, pattern=[[1, KT]], channel_multiplier=-1,
            )
            nc.vector.tensor_copy(tri1[:], trF[:, 0:KT])
            nc.gpsimd.memset(trF[:], 1.0)
            nc.gpsimd.affine_select(
                out=trF[:], in_=trF[:], compare_op=IS_GE,
                fill=0.0, base=-KT, pattern=[[1, 2 * KT]], channel_multiplier=-1,
            )
            nc.vector.tensor_copy(tri2[:], trF[:])

        # ---- persistent qkT / v storage --------------------------------
        # A=[q0|q1] B=[k0|k1] C=[q2|k2] C'=[k2|q2]
        qkt = [qkp.tile([P, t], BF16, tag=f"qkt{i}", name=f"qkt{i}")
               for i in range(4)]
        A, Bt, Ct, Cp = qkt
        vnat = [vp.tile([P, NVP], BF16, tag=f"vn{j}", name=f"vn{j}")
                for j in range(t // P)]

        def v1ap(h, j):
            return vnat[j][:, 65 * h:65 * h + DH + 1]

        # ---- deferred PE-side work queue -------------------------------
        pe_q = deque()

        def pump(n=1):
            for _ in range(n):
                if not pe_q:
                    return
                pe_q.popleft()()

        # ---- per-group QKV emission ------------------------------------
        xtiles = {}

        def emit_qkv_unit(g, u):
            gs = slice(g * QG, (g + 1) * QG)
            if u == 0:
                xa = xpool.tile([P, 6 * QG], BF16, tag="xa", name="xa")
                src3 = xT[:, :].rearrange("p (k t) -> p k t", k=6)[:, :,
                                                                  gs]
                dst3 = xa[:].rearrange("p (k q) -> p k q", k=6)
                nc.sync.dma_start(dst3, src3)
                xtiles[g] = xa
                return
            xa = xtiles[g]

            def xk(k):
                return xa[:, k * QG:(k + 1) * QG]
            if u == 4:
                # C' = [k2|q2]: partition-swapped copy of C on the (slack)
                # scalar engine instead of 6 more PE matmuls.
                nc.scalar.copy(Cp[0:DH, gs], Ct[DH:P, gs])
                nc.scalar.copy(Cp[DH:P, gs], Ct[0:DH, gs])
            elif u <= 3:
                m = u - 1           # qkT m-tile (A, B, C)
                ps = aux.tile([P, QG], F32, tag="aux", name="qkps")
                for k in range(6):
                    nc.tensor.matmul(ps[:], wqk_sb[k][:, m * P:(m + 1) * P],
                                     xk(k), start=(k == 0), stop=(k == 5))
                nc.vector.tensor_scalar_add(qkt[m][:, gs], ps[:], b_sb[m])
            else:
                ti = u - 5          # v t-tile within the group (0..3)
                j = 4 * g + ti
                ps = aux.tile([P, NVP], F32, tag="aux", name="vnps")
                for k in range(6):
                    nc.tensor.matmul(ps[:], xk(k)[:, ti * P:(ti + 1) * P],
                                     wvp_sb[k], start=(k == 0), stop=False)
                nc.tensor.matmul(ps[:], ones1t[:], vpat[:],
                                 start=False, stop=True)
                nc.vector.tensor_copy(vnat[j][:], ps[:])
                if u == 8:
                    xtiles.pop(g)   # release python ref (slots reused by tag)

        N_UNITS = 9  # 1 dma + 4 qk + 4 v

        # ---- attention unit (2 S^T blocks -> exp -> 2 PV) --------------
        def emit_unit(ytA, ytB, lhsA, lhsB, rhsA, rhsB, jA, jB, hA, hB,
                      w, qoff, rA, rB, startA, stopA, startB, stopB):
            """One unit: two S^T matmuls of width w at s2 cols 0 / 512,
            one exp over both blocks, triangle masks if diagonal, two PV
            matmuls accumulating into ytA/ytB cols [qoff:512].  rA/rB:
            None = no mask, >=0 = tri1 at block start, -1 = tri2 (block
            carries an extra 128 fully-masked columns; phase-B 2nd tile).
            """
            s2 = spsum.tile([P, 2 * QG], F32, tag="s", name="s")
            nc.tensor.matmul(s2[:, 0:w], lhsA, rhsA, start=True, stop=True)
            nc.tensor.matmul(s2[:, QG:QG + w], lhsB, rhsB,
                             start=True, stop=True)
            # Idempotent ballast: re-issue the S pair (same output, full
            # rewrite).  Keeps the PE array dense when no deferred work is
            # available, so the HAM clock gate stays at 8/8; the rewrite
            # produces identical values, and exp simply waits for the last.
            for _ in range(ECHO if not pe_q else 0):
                nc.tensor.matmul(s2[:, 0:w], lhsA, rhsA,
                                 start=True, stop=True)
                nc.tensor.matmul(s2[:, QG:QG + w], lhsB, rhsB,
                                 start=True, stop=True)
            p2 = ppool.tile([P, 2 * QG], BF16, tag="p", name="p")
            if w == QG:
                nc.scalar.activation(p2[:, 0:2 * QG], s2[:, 0:2 * QG], EXP,
                                     scale=float(SCALE))
            elif os.environ.get("K_NO_REARRANGE"):
                nc.scalar.activation(p2[:, 0:w], s2[:, 0:w], EXP,
                                     scale=float(SCALE))
                nc.scalar.activation(p2[:, QG:QG + w], s2[:, QG:QG + w], EXP,
                                     scale=float(SCALE))
            else:
                s3 = s2[:].rearrange("p (a b) -> p a b", a=2)[:, :, 0:w]
                p3 = p2[:].rearrange("p (a b) -> p a b", a=2)[:, :, 0:w]
                nc.scalar.activation(p3, s3, EXP, scale=float(SCALE))
            if rA is not None:
                nc.vector.tensor_mul(p2[:, 0:KT], p2[:, 0:KT], tri1[:])
            if rB is not None:
                if rB >= 0:
                    nc.vector.tensor_mul(p2[:, QG:QG + KT], p2[:, QG:QG + KT],
                                         tri1[:])
                else:
                    nc.vector.tensor_mul(p2[:, QG:QG + 2 * KT],
                                         p2[:, QG:QG + 2 * KT], tri2[:])
            nc.tensor.matmul(ytA[:, qoff:QG], v1ap(hA, jA), p2[:, 0:w],
                             start=startA, stop=stopA)
            nc.tensor.matmul(ytB[:, qoff:QG], v1ap(hB, jB),
                             p2[:, QG:QG + w],
                             start=startB, stop=stopB)

        # ---- epilogue --------------------------------------------------
        def emit_epilogue(g, h, yt_ps, dest):
            ysb = ysbp.tile([DH + 1, QG], F32, tag="ysb", name="ysb")
            nc.vector.tensor_copy(ysb[:], yt_ps[:])
            # reciprocal_approx_fast corrupts when its input AP sits at a
            # nonzero base partition: bounce the denominator to partition 0.
            d_f = rrp.tile([1, QG], F32, tag="df", name="df")
            nc.vector.tensor_copy(d_f[:], ysb[DH:DH + 1, :])
            r_f = rrp.tile([1, QG], F32, tag="rf", name="rf")
            nc.vector.reciprocal_approx_fast(r_f[:], d_f[:])
            r_r = rrp.tile([1, QG], BF16, tag="rr", name="rr")
            nc.vector.tensor_copy(r_r[:], r_f[:])

            def finish():
                R_t = aux.tile([DH, QG], F32, tag="aux", name="Rb")
                nc.tensor.matmul(R_t[:], ones1[:], r_r[:],
                                 start=True, stop=True)
                nc.vector.scalar_tensor_tensor(
                    out=dest, in0=R_t[:], scalar=1.0, in1=ysb[0:DH, :],
                    op0=MUL, op1=MUL)
            pe_q.append(finish)

        # ---- output projection (deferred) ------------------------------
        def push_proj(g, y0, y1):
            gs = slice(g * QG, (g + 1) * QG)

            def mk(cm):
                def run():
                    op = aux.tile([P, QG], F32, tag="aux", name="oo")
                    nc.tensor.matmul(op[:], wp0[:, cm * P:(cm + 1) * P],
                                     y0[:], start=True, stop=False)
                    nc.tensor.matmul(op[:], wp1[:, cm * P:(cm + 1) * P],
                                     y1[:], start=False, stop=True)
                    ob = osb.tile([P, QG], F32, tag="ob", name="ob")
                    nc.vector.tensor_copy(ob[:], op[:])
                    nc.sync.dma_start(outT[cm * P:(cm + 1) * P, gs], ob[:])
                return run
            for cm in range(6):
                pe_q.append(mk(cm))

        # ---- prologue: group 0's QKV -----------------------------------
        for u in range(N_UNITS):
            emit_qkv_unit(0, u)

        # ---- main loop --------------------------------------------------
        for g in range(ng):
            nkt = 4 * (g + 1)    # k-tiles this group
            gq0 = g * QG
            if g + 1 < ng:
                for u in range(N_UNITS):
                    uu = u
                    pe_q.append(lambda gg=g + 1, uu=uu: emit_qkv_unit(gg, uu))

            n_slots = 6 * (g + 1) + 3
            slot = 0
            popped = 0

            def pump(n=1):
                nonlocal popped
                for _ in range(n):
                    if not pe_q:
                        return
                    pe_q.popleft()()
                    popped += 1

            def pump_slot():
                # uniform spread of deferred work across the whole group so
                # fills remain available to cover late phase-boundary stalls
                nonlocal slot
                slot += 1
                if slot >= n_slots:
                    pump(len(pe_q))
                else:
                    total = popped + len(pe_q)
                    want = total * slot // n_slots
                    if want > popped:
                        pump(want - popped)

            yt01 = [ytps.tile([DH + 1, QG], F32, tag="yt", name="yt0p"),
                    ytps.tile([DH + 1, QG], F32, tag="yt", name="yt1p")]
            y0 = ytsb.tile([P, QG], BF16, tag="yt0", name="yt0")
            y1 = ytsb.tile([DH, QG], BF16, tag="yt1", name="yt1")

            # ---- phase A: heads 0 & 1, one k-tile per unit -------------
            for j in range(nkt):
                r = j - 4 * g
                w = QG if r < 0 else QG - KT * r
                qoff = QG - w
                tc0 = j * KT
                qs = slice(gq0 + qoff, gq0 + QG)
                emit_unit(
                    yt01[0], yt01[1],
                    Bt[0:DH, tc0:tc0 + KT], Bt[DH:P, tc0:tc0 + KT],
                    A[0:DH, qs], A[DH:P, qs],
                    j, j, 0, 1, w, qoff,
                    (r if r >= 0 else None), (r if r >= 0 else None),
                    startA=(j == 0), stopA=(j == nkt - 1),
                    startB=(j == 0), stopB=(j == nkt - 1))
                pump_slot()
            emit_epilogue(g, 0, yt01[0], y0[0:DH, :])
            pump_slot()
            emit_epilogue(g, 1, yt01[1], y0[DH:P, :])
            pump_slot()

            # ---- phase B: head 2, two k-tiles per unit -----------------
            yt2 = ytps.tile([DH + 1, QG], F32, tag="yt", name="yt2p")
            npr = 2 * (g + 1)
            for pr in range(npr):
                j0, j1 = 2 * pr, 2 * pr + 1
                r0 = j0 - 4 * g
                w0 = QG if r0 < 0 else QG - KT * r0
                qoff = QG - w0
                qs = slice(gq0 + qoff, gq0 + QG)
                emit_unit(
                    yt2, yt2,
                    Cp[0:DH, j0 * KT:(j0 + 1) * KT],
                    Ct[DH:P, j1 * KT:(j1 + 1) * KT],
                    Ct[0:DH, qs], Cp[DH:P, qs],
                    j0, j1, 2, 2, w0, qoff,
                    (r0 if r0 >= 0 else None),
                    (-1 if r0 >= 0 else None),   # -1 -> tri2 on block B
                    startA=(pr == 0), stopA=False,
                    startB=False, stopB=(pr == npr - 1))
                pump_slot()
            emit_epilogue(g, 2, yt2, y1[:])
            pump_slot()

            push_proj(g, y0, y1)

        while pe_q:
            pump()
    nc.compile()
    return nc


_NC_CACHE = {}


def get_nc(t=T_FULL):
    if t not in _NC_CACHE:
        _NC_CACHE[t] = build_nc(t)
    return _NC_CACHE[t]


def make_in_maps(x, W_attn, b_attn, W_proj):
    import ml_dtypes
    bf16 = ml_dtypes.bfloat16
    x = np.ascontiguousarray(np.asarray(x, np.float32))
    W_attn = np.asarray(W_attn, np.float32)
    b_attn = np.asarray(b_attn, np.float32)
    W_proj = np.asarray(W_proj, np.float32)

    def qcol(h):
        return W_attn[:, h * DH:(h + 1) * DH]

    def kcol(h):
        return W_attn[:, C + h * DH:C + (h + 1) * DH]

    def qb(h):
        return b_attn[h * DH:(h + 1) * DH]

    def kb(h):
        return b_attn[C + h * DH:C + (h + 1) * DH]

    in_maps = []
    for c in range(NCORES):
        b = c // 4
        hs = [3 * (c % 4) + i for i in range(HPC)]
        # A=[q0|q1] B=[k0|k1] C=[q2|k2] C'=[k2|q2]
        cols = [qcol(hs[0]), qcol(hs[1]), kcol(hs[0]), kcol(hs[1]),
                qcol(hs[2]), kcol(hs[2]), kcol(hs[2]), qcol(hs[2])]
        wqk = np.ascontiguousarray(np.concatenate(cols, axis=1))
        bqk = np.concatenate(
            [qb(hs[0]), qb(hs[1]), kb(hs[0]), kb(hs[1]),
             qb(hs[2]), kb(hs[2]), kb(hs[2]), qb(hs[2])]
        ).reshape(NQK, 1)
        wvp = np.zeros((C, NVP), np.float32)
        for lh in range(HPC):
            wvp[:, 65 * lh:65 * lh + DH] = \
                W_attn[:, 2 * C + hs[lh] * DH:2 * C + (hs[lh] + 1) * DH]
        wp = np.ascontiguousarray(
            np.concatenate([W_proj[h * DH:(h + 1) * DH, :] for h in hs], axis=0)
        )
        xTc = x[b].T                      # [C, t]
        t = xTc.shape[1]
        xTp = np.concatenate([xTc[k * P:(k + 1) * P, :] for k in range(6)],
                             axis=1)      # [128, 6t]
        wqkp = np.concatenate([wqk[k * P:(k + 1) * P, :] for k in range(6)],
                              axis=1)     # [128, 6*NQK]
        wvpp = np.concatenate([wvp[k * P:(k + 1) * P, :] for k in range(6)],
                              axis=1)     # [128, 6*NVP]
        bqkp = bqk.reshape(4, P).T        # [128, 4]
        in_maps.append({
            "xT": np.ascontiguousarray(xTp).astype(bf16),
            "wqk": np.ascontiguousarray(wqkp).astype(bf16),
            "bqk": np.ascontiguousarray(bqkp, dtype=np.float32),
            "wvp": np.ascontiguousarray(wvpp).astype(bf16),
            "wp": wp.astype(bf16),
        })
    return in_maps


def unshard(per_core_outT, bias_vec):
    t = per_core_outT[0].shape[1]
    out = np.zeros((B, t, C), np.float32)
    for c in range(NCORES):
        out[c // 4] += per_core_outT[c].T
    out += np.asarray(bias_vec, np.float32)[None, None, :]
    return out


def kernel(x, W_attn, b_attn, W_proj, b_proj, **run_kwargs):
    nc = get_nc(T_FULL)
    in_maps = make_in_maps(x, W_attn, b_attn, W_proj)
    # v-bias and b_proj fold into one per-channel constant:
    # out = sum_h Wp_h^T (attn_h) + (b_v @ W_proj + b_proj)
    b_attn = np.asarray(b_attn, np.float32)
    bias_vec = b_attn[2 * C:] @ np.asarray(W_proj, np.float32) \
        + np.asarray(b_proj, np.float32)
    res = None
    last_err = None
    for attempt in range(3):
        try:
            res = run_bass_kernel_spmd(nc, in_maps,
                                       core_ids=list(range(NCORES)),
                                       **run_kwargs)
            break
        except Exception as e:  # transient NRT_EXEC_UNIT_UNRECOVERABLE etc.
            last_err = e
    if res is None:
        raise last_err
    outs = [res.results[c]["outT"] for c in range(NCORES)]
    out = unshard(outs, bias_vec)
    return out


# revision 20
# speedup vs baseline: 1.1629x; 1.1629x over previous
"""Causal self-attention (B=2, T=4096, C=768, H=12) on 8 TRN2 NeuronCores.

Sharding: core c -> batch c//4, heads 3*(c%4) .. 3*(c%4)+2.  Each core is
fully independent (no collectives): it computes qkv for its 3 heads from
x[b], runs causal flash attention, and produces the partial output
projection outT = (Y_heads @ W_proj[rows]).T of shape [C, T].  The host
sums the 4 per-batch partials, transposes, and adds the folded bias
(W_proj^T b_v + b_proj).

Per-core structure (all matmuls float32r, 1 cyc/row at N>=256):
  - qkT kept as 4 m-tiles A=[q0|q1], B=[k0|k1], C=[q2|k2], C'=[k2|q2]
    (host-packed weight layout) so every S^T matmul pair issues from PE
    row-groups {0,1} and {2,3} concurrently with NO duplicate copies:
    heads 0/1 pair across the partition halves of A/B; head 2 uses C/C'.
  - v in natural [T, 64] orientation with a ones column (softmax
    denominator rides in the PV matmul); the ones are injected by a K=1
    matmul against a constant pattern row, not DVE writes.
  - exp on ACT with no max subtraction (logits are bounded); diagonal
    k-tiles are column-compacted (only q >= k block computed) and masked
    with small [128,128/256] triangle multiplies on DVE.
  - softmax division: PSUM yt -> SBUF copy (frees the PSUM bank for the
    next group immediately), reciprocal_approx_fast, K=1 broadcast
    matmul, and one fused scalar_tensor_tensor multiply.
  - All side work (next group's QKV, previous group's output projection,
    epilogue broadcast/divide) flows through a deferred-thunk queue that
    is pumped between attention units, keeping the PE stream dense so
    the HAM clock gate stays at 8/8.
"""

import os
import sys

import numpy as np

for _p in ("/opt/trn_rl_repo", "/root/.axon_site/_ro/trn_rl_repo"):
    if os.path.isdir(_p) and _p not in sys.path:
        sys.path.insert(0, _p)

from collections import deque
from contextlib import ExitStack

import concourse.bacc as bacc
import concourse.bass as bass
import concourse.mybir as mybir
import concourse.tile as tile
from concourse.bass_utils import run_bass_kernel_spmd

F32 = mybir.dt.float32
F32R = mybir.dt.float32r
BF16 = mybir.dt.bfloat16
EXP = mybir.ActivationFunctionType.Exp
IS_GE = mybir.AluOpType.is_ge
MUL = mybir.AluOpType.mult

ECHO = int(os.environ.get("K_ECHO", "0"))

B, T_FULL, C = 2, 4096, 768
H, DH = 12, 64
HPC = 3                      # heads per core
NCORES = 8
P = 128
QG = 512                     # query-group span
KT = 128                     # key tile
NQK = 4 * P                  # 512 rows of qkT (A, B, C, C')
NVP = 256                    # padded v width: v0|1|v1|1|v2|1|zeros
SCALE = 1.0 / np.sqrt(DH)


def r32(ap):
    return ap.bitcast(F32R)


def build_nc(t=T_FULL):
    ng = t // QG             # query groups
    nc = bacc.Bacc(None, target_bir_lowering=False)
    xT = nc.declare_dram_parameter("xT", [P, 6 * t], BF16, isOutput=False)
    wqk = nc.declare_dram_parameter("wqk", [P, 6 * NQK], BF16, isOutput=False)
    bqk = nc.declare_dram_parameter("bqk", [P, 4], F32, isOutput=False)
    wvp = nc.declare_dram_parameter("wvp", [P, 6 * NVP], BF16, isOutput=False)
    wp = nc.declare_dram_parameter("wp", [HPC * DH, C], BF16, isOutput=False)
    outT = nc.declare_dram_parameter("outT", [C, t], F32, isOutput=True)

    with tile.TileContext(nc) as tc, ExitStack() as ctx:
        const = ctx.enter_context(tc.tile_pool(name="const", bufs=1))
        qkp = ctx.enter_context(tc.tile_pool(name="qk", bufs=1))
        vp = ctx.enter_context(tc.tile_pool(name="vn", bufs=1))
        xpool = ctx.enter_context(tc.tile_pool(name="xin", bufs=1))
        ppool = ctx.enter_context(tc.tile_pool(name="pp", bufs=3))
        ytsb = ctx.enter_context(tc.tile_pool(name="ytsb", bufs=2))
        ysbp = ctx.enter_context(tc.tile_pool(name="ysb", bufs=3))
        rrp = ctx.enter_context(tc.tile_pool(name="rr", bufs=3))
        osb = ctx.enter_context(tc.tile_pool(name="osb", bufs=2))
        spsum = ctx.enter_context(tc.tile_pool(name="sps", bufs=2, space="PSUM"))
        ytps = ctx.enter_context(tc.tile_pool(name="ytps", bufs=2, space="PSUM"))
        aux = ctx.enter_context(tc.tile_pool(name="aux", bufs=2, space="PSUM"))

        # ---- constants (packed single-DMA loads) -----------------------
        wqk_all = const.tile([P, 6 * NQK], BF16, tag="wqka", name="wqka")
        nc.sync.dma_start(wqk_all[:], wqk[:, :])
        wqk_sb = [wqk_all[:, k * NQK:(k + 1) * NQK] for k in range(6)]
        wvp_all = const.tile([P, 6 * NVP], BF16, tag="wvpa", name="wvpa")
        nc.sync.dma_start(wvp_all[:], wvp[:, :])
        wvp_sb = [wvp_all[:, k * NVP:(k + 1) * NVP] for k in range(6)]
        bq_all = const.tile([P, 4], F32, tag="bqa", name="bqa")
        nc.sync.dma_start(bq_all[:], bqk[:, :])
        b_sb = [bq_all[:, m:m + 1] for m in range(4)]
        wp0 = const.tile([P, C], BF16, tag="wp0", name="wp0")
        wp1 = const.tile([DH, C], BF16, tag="wp1", name="wp1")
        nc.sync.dma_start(wp0[:], wp[0:P, :])
        nc.sync.dma_start(wp1[:], wp[P:P + DH, :])

        ones1 = const.tile([1, DH], BF16, tag="ones1", name="ones1")
        ones1t = const.tile([1, P], BF16, tag="ones1t", name="ones1t")
        vpat = const.tile([1, NVP], BF16, tag="vpat", name="vpat")
        tri1 = const.tile([P, KT], BF16, tag="tri1", name="tri1")
        tri2 = const.tile([P, 2 * KT], BF16, tag="tri2", name="tri2")
        with tc.tile_pool(name="scratch", bufs=1) as scratch:
            onesRF = scratch.tile([1, P], F32, tag="onesRF", name="onesRF")
            nc.vector.memset(onesRF[:], 1.0)
            nc.vector.tensor_copy(ones1[:], onesRF[:, 0:DH])
            nc.vector.tensor_copy(ones1t[:], onesRF[:])
            vpF = scratch.tile([1, NVP], F32, tag="vpF", name="vpF")
            nc.vector.memset(vpF[:], 0.0)
            for h in range(HPC):
                nc.vector.memset(vpF[:, 65 * h + DH:65 * h + DH + 1], 1.0)
            nc.vector.tensor_copy(vpat[:], vpF[:])
            # tri1[k, q] = 1 iff q >= k ; tri2[k, q] = 1 iff q >= k + 128
            trF = scratch.tile([P, 2 * KT], F32, tag="trF", name="trF")
            nc.gpsimd.memset(trF[:], 1.0)
            nc.gpsimd.affine_select(
                out=trF[:, 0:KT], in_=trF[:, 0:KT], compare_op=IS_GE,
                fill=0.0, base=0, pattern=[[1, KT]], channel_multiplier=-1,
            )
            nc.vector.tensor_copy(tri1[:], trF[:, 0:KT])
            nc.gpsimd.memset(trF[:], 1.0)
            nc.gpsimd.affine_select(
                out=trF[:], in_=trF[:], compare_op=IS_GE,
                fill=0.0, base=-KT, pattern=[[1, 2 * KT]], channel_multiplier=-1,
            )
            nc.vector.tensor_copy(tri2[:], trF[:])

        # ---- persistent qkT / v storage --------------------------------
        # A=[q0|q1] B=[k0|k1] C=[q2|k2] C'=[k2|q2]
        qkt = [qkp.tile([P, t], BF16, tag=f"qkt{i}", name=f"qkt{i}")
               for i in range(4)]
        A, Bt, Ct, Cp = qkt
        vnat = [vp.tile([P, NVP], BF16, tag=f"vn{j}", name=f"vn{j}")
                for j in range(t // P)]

        def v1ap(h, j):
            return vnat[j][:, 65 * h:65 * h + DH + 1]

        # ---- deferred PE-side work queue -------------------------------
        pe_q = deque()

        def pump(n=1):
            for _ in range(n):
                if not pe_q:
                    return
                pe_q.popleft()()

        # ---- per-group QKV emission ------------------------------------
        xtiles = {}

        def emit_qkv_unit(g, u):
            gs = slice(g * QG, (g + 1) * QG)
            if u == 0:
                xa = xpool.tile([P, 6 * QG], BF16, tag="xa", name="xa")
                src3 = xT[:, :].rearrange("p (k t) -> p k t", k=6)[:, :,
                                                                  gs]
                dst3 = xa[:].rearrange("p (k q) -> p k q", k=6)
                nc.sync.dma_start(dst3, src3)
                xtiles[g] = xa
                return
            xa = xtiles[g]

            def xk(k):
                return xa[:, k * QG:(k + 1) * QG]
            if u <= 4:
                m = u - 1           # qkT m-tile (A, B, C, C')
                ps = aux.tile([P, QG], F32, tag="aux", name="qkps")
                for k in range(6):
                    nc.tensor.matmul(ps[:], wqk_sb[k][:, m * P:(m + 1) * P],
                                     xk(k), start=(k == 0), stop=(k == 5))
                nc.vector.tensor_scalar_add(qkt[m][:, gs], ps[:], b_sb[m])
            else:
                ti = u - 5          # v t-tile within the group (0..3)
                j = 4 * g + ti
                ps = aux.tile([P, NVP], F32, tag="aux", name="vnps")
                for k in range(6):
                    nc.tensor.matmul(ps[:], xk(k)[:, ti * P:(ti + 1) * P],
                                     wvp_sb[k], start=(k == 0), stop=False)
                nc.tensor.matmul(ps[:], ones1t[:], vpat[:],
                                 start=False, stop=True)
                nc.vector.tensor_copy(vnat[j][:], ps[:])
                if u == 8:
                    xtiles.pop(g)   # release python ref (slots reused by tag)

        N_UNITS = 9  # 1 dma + 4 qk + 4 v

        # ---- attention unit (2 S^T blocks -> exp -> 2 PV) --------------
        def emit_unit(ytA, ytB, lhsA, lhsB, rhsA, rhsB, jA, jB, hA, hB,
                      w, qoff, rA, rB, startA, stopA, startB, stopB):
            """One unit: two S^T matmuls of width w at s2 cols 0 / 512,
            one exp over both blocks, triangle masks if diagonal, two PV
            matmuls accumulating into ytA/ytB cols [qoff:512].  rA/rB:
            None = no mask, >=0 = tri1 at block start, -1 = tri2 (block
            carries an extra 128 fully-masked columns; phase-B 2nd tile).
            """
            s2 = spsum.tile([P, 2 * QG], F32, tag="s", name="s")
            nc.tensor.matmul(s2[:, 0:w], lhsA, rhsA, start=True, stop=True)
            nc.tensor.matmul(s2[:, QG:QG + w], lhsB, rhsB,
                             start=True, stop=True)
            # Idempotent ballast: re-issue the S pair (same output, full
            # rewrite).  Keeps the PE array dense when no deferred work is
            # available, so the HAM clock gate stays at 8/8; the rewrite
            # produces identical values, and exp simply waits for the last.
            for _ in range(ECHO if not pe_q else 0):
                nc.tensor.matmul(s2[:, 0:w], lhsA, rhsA,
                                 start=True, stop=True)
                nc.tensor.matmul(s2[:, QG:QG + w], lhsB, rhsB,
                                 start=True, stop=True)
            p2 = ppool.tile([P, 2 * QG], BF16, tag="p", name="p")
            if w == QG:
                nc.scalar.activation(p2[:, 0:2 * QG], s2[:, 0:2 * QG], EXP,
                                     scale=float(SCALE))
            elif os.environ.get("K_NO_REARRANGE"):
                nc.scalar.activation(p2[:, 0:w], s2[:, 0:w], EXP,
                                     scale=float(SCALE))
                nc.scalar.activation(p2[:, QG:QG + w], s2[:, QG:QG + w], EXP,
                                     scale=float(SCALE))
            else:
                s3 = s2[:].rearrange("p (a b) -> p a b", a=2)[:, :, 0:w]
                p3 = p2[:].rearrange("p (a b) -> p a b", a=2)[:, :, 0:w]
                nc.scalar.activation(p3, s3, EXP, scale=float(SCALE))
            if rA is not None:
                nc.vector.tensor_mul(p2[:, 0:KT], p2[:, 0:KT], tri1[:])
            if rB is not None:
                if rB >= 0:
                    nc.vector.tensor_mul(p2[:, QG:QG + KT], p2[:, QG:QG + KT],
                                         tri1[:])
                else:
                    nc.vector.tensor_mul(p2[:, QG:QG + 2 * KT],
                                         p2[:, QG:QG + 2 * KT], tri2[:])
            nc.tensor.matmul(ytA[:, qoff:QG], v1ap(hA, jA), p2[:, 0:w],
                             start=startA, stop=stopA)
            nc.tensor.matmul(ytB[:, qoff:QG], v1ap(hB, jB),
                             p2[:, QG:QG + w],
                             start=startB, stop=stopB)

        # ---- epilogue --------------------------------------------------
        def emit_epilogue(g, h, yt_ps, dest):
            ysb = ysbp.tile([DH + 1, QG], F32, tag="ysb", name="ysb")
            nc.vector.tensor_copy(ysb[:], yt_ps[:])
            # reciprocal_approx_fast corrupts when its input AP sits at a
            # nonzero base partition: bounce the denominator to partition 0.
            d_f = rrp.tile([1, QG], F32, tag="df", name="df")
            nc.vector.tensor_copy(d_f[:], ysb[DH:DH + 1, :])
            r_f = rrp.tile([1, QG], F32, tag="rf", name="rf")
            nc.vector.reciprocal_approx_fast(r_f[:], d_f[:])
            r_r = rrp.tile([1, QG], BF16, tag="rr", name="rr")
            nc.vector.tensor_copy(r_r[:], r_f[:])

            def finish():
                R_t = aux.tile([DH, QG], F32, tag="aux", name="Rb")
                nc.tensor.matmul(R_t[:], ones1[:], r_r[:],
                                 start=True, stop=True)
                nc.vector.scalar_tensor_tensor(
                    out=dest, in0=R_t[:], scalar=1.0, in1=ysb[0:DH, :],
                    op0=MUL, op1=MUL)
            pe_q.append(finish)

        # ---- output projection (deferred) ------------------------------
        def push_proj(g, y0, y1):
            gs = slice(g * QG, (g + 1) * QG)

            def mk(cm):
                def run():
                    op = aux.tile([P, QG], F32, tag="aux", name="oo")
                    nc.tensor.matmul(op[:], wp0[:, cm * P:(cm + 1) * P],
                                     y0[:], start=True, stop=False)
                    nc.tensor.matmul(op[:], wp1[:, cm * P:(cm + 1) * P],
                                     y1[:], start=False, stop=True)
                    ob = osb.tile([P, QG], F32, tag="ob", name="ob")
                    nc.vector.tensor_copy(ob[:], op[:])
                    nc.sync.dma_start(outT[cm * P:(cm + 1) * P, gs], ob[:])
                return run
            for cm in range(6):
                pe_q.append(mk(cm))

        # ---- prologue: group 0's QKV -----------------------------------
        for u in range(N_UNITS):
            emit_qkv_unit(0, u)

        # ---- main loop --------------------------------------------------
        for g in range(ng):
            nkt = 4 * (g + 1)    # k-tiles this group
            gq0 = g * QG
            if g + 1 < ng:
                for u in range(N_UNITS):
                    uu = u
                    pe_q.append(lambda gg=g + 1, uu=uu: emit_qkv_unit(gg, uu))

            n_slots = 6 * (g + 1) + 3
            slot = 0
            popped = 0

            def pump(n=1):
                nonlocal popped
                for _ in range(n):
                    if not pe_q:
                        return
                    pe_q.popleft()()
                    popped += 1

            def pump_slot():
                # uniform spread of deferred work across the whole group so
                # fills remain available to cover late phase-boundary stalls
                nonlocal slot
                slot += 1
                if slot >= n_slots:
                    pump(len(pe_q))
                else:
                    total = popped + len(pe_q)
                    want = total * slot // n_slots
                    if want > popped:
                        pump(want - popped)

            yt01 = [ytps.tile([DH + 1, QG], F32, tag="yt", name="yt0p"),
                    ytps.tile([DH + 1, QG], F32, tag="yt", name="yt1p")]
            y0 = ytsb.tile([P, QG], BF16, tag="yt0", name="yt0")
            y1 = ytsb.tile([DH, QG], BF16, tag="yt1", name="yt1")

            # ---- phase A: heads 0 & 1, one k-tile per unit -------------
            for j in range(nkt):
                r = j - 4 * g
                w = QG if r < 0 else QG - KT * r
                qoff = QG - w
                tc0 = j * KT
                qs = slice(gq0 + qoff, gq0 + QG)
                emit_unit(
                    yt01[0], yt01[1],
                    Bt[0:DH, tc0:tc0 + KT], Bt[DH:P, tc0:tc0 + KT],
                    A[0:DH, qs], A[DH:P, qs],
                    j, j, 0, 1, w, qoff,
                    (r if r >= 0 else None), (r if r >= 0 else None),
                    startA=(j == 0), stopA=(j == nkt - 1),
                    startB=(j == 0), stopB=(j == nkt - 1))
                pump_slot()
            emit_epilogue(g, 0, yt01[0], y0[0:DH, :])
            pump_slot()
            emit_epilogue(g, 1, yt01[1], y0[DH:P, :])
            pump_slot()

            # ---- phase B: head 2, two k-tiles per unit -----------------
            yt2 = ytps.tile([DH + 1, QG], F32, tag="yt", name="yt2p")
            npr = 2 * (g + 1)
            for pr in range(npr):
                j0, j1 = 2 * pr, 2 * pr + 1
                r0 = j0 - 4 * g
                w0 = QG if r0 < 0 else QG - KT * r0
                qoff = QG - w0
                qs = slice(gq0 + qoff, gq0 + QG)
                emit_unit(
                    yt2, yt2,
                    Cp[0:DH, j0 * KT:(j0 + 1) * KT],
                    Ct[DH:P, j1 * KT:(j1 + 1) * KT],
                    Ct[0:DH, qs], Cp[DH:P, qs],
                    j0, j1, 2, 2, w0, qoff,
                    (r0 if r0 >= 0 else None),
                    (-1 if r0 >= 0 else None),   # -1 -> tri2 on block B
                    startA=(pr == 0), stopA=False,
                    startB=False, stopB=(pr == npr - 1))
                pump_slot()
            emit_epilogue(g, 2, yt2, y1[:])
            pump_slot()

            push_proj(g, y0, y1)

        while pe_q:
            pump()
    nc.compile()
    return nc


_NC_CACHE = {}


def get_nc(t=T_FULL):
    if t not in _NC_CACHE:
        _NC_CACHE[t] = build_nc(t)
    return _NC_CACHE[t]


def make_in_maps(x, W_attn, b_attn, W_proj):
    import ml_dtypes
    bf16 = ml_dtypes.bfloat16
    x = np.ascontiguousarray(np.asarray(x, np.float32))
    W_attn = np.asarray(W_attn, np.float32)
    b_attn = np.asarray(b_attn, np.float32)
    W_proj = np.asarray(W_proj, np.float32)

    def qcol(h):
        return W_attn[:, h * DH:(h + 1) * DH]

    def kcol(h):
        return W_attn[:, C + h * DH:C + (h + 1) * DH]

    def qb(h):
        return b_attn[h * DH:(h + 1) * DH]

    def kb(h):
        return b_attn[C + h * DH:C + (h + 1) * DH]

    in_maps = []
    for c in range(NCORES):
        b = c // 4
        hs = [3 * (c % 4) + i for i in range(HPC)]
        # A=[q0|q1] B=[k0|k1] C=[q2|k2] C'=[k2|q2]
        cols = [qcol(hs[0]), qcol(hs[1]), kcol(hs[0]), kcol(hs[1]),
                qcol(hs[2]), kcol(hs[2]), kcol(hs[2]), qcol(hs[2])]
        wqk = np.ascontiguousarray(np.concatenate(cols, axis=1))
        bqk = np.concatenate(
            [qb(hs[0]), qb(hs[1]), kb(hs[0]), kb(hs[1]),
             qb(hs[2]), kb(hs[2]), kb(hs[2]), qb(hs[2])]
        ).reshape(NQK, 1)
        wvp = np.zeros((C, NVP), np.float32)
        for lh in range(HPC):
            wvp[:, 65 * lh:65 * lh + DH] = \
                W_attn[:, 2 * C + hs[lh] * DH:2 * C + (hs[lh] + 1) * DH]
        wp = np.ascontiguousarray(
            np.concatenate([W_proj[h * DH:(h + 1) * DH, :] for h in hs], axis=0)
        )
        xTc = x[b].T                      # [C, t]
        t = xTc.shape[1]
        xTp = np.concatenate([xTc[k * P:(k + 1) * P, :] for k in range(6)],
                             axis=1)      # [128, 6t]
        wqkp = np.concatenate([wqk[k * P:(k + 1) * P, :] for k in range(6)],
                              axis=1)     # [128, 6*NQK]
        wvpp = np.concatenate([wvp[k * P:(k + 1) * P, :] for k in range(6)],
                              axis=1)     # [128, 6*NVP]
        bqkp = bqk.reshape(4, P).T        # [128, 4]
        in_maps.append({
            "xT": np.ascontiguousarray(xTp).astype(bf16),
            "wqk": np.ascontiguousarray(wqkp).astype(bf16),
            "bqk": np.ascontiguousarray(bqkp, dtype=np.float32),
            "wvp": np.ascontiguousarray(wvpp).astype(bf16),
            "wp": wp.astype(bf16),
        })
    return in_maps


def unshard(per_core_outT, bias_vec):
    t = per_core_outT[0].shape[1]
    out = np.zeros((B, t, C), np.float32)
    for c in range(NCORES):
        out[c // 4] += per_core_outT[c].T
    out += np.asarray(bias_vec, np.float32)[None, None, :]
    return out


def kernel(x, W_attn, b_attn, W_proj, b_proj, **run_kwargs):
    nc = get_nc(T_FULL)
    in_maps = make_in_maps(x, W_attn, b_attn, W_proj)
    # v-bias and b_proj fold into one per-channel constant:
    # out = sum_h Wp_h^T (attn_h) + (b_v @ W_proj + b_proj)
    b_attn = np.asarray(b_attn, np.float32)
    bias_vec = b_attn[2 * C:] @ np.asarray(W_proj, np.float32) \
        + np.asarray(b_proj, np.float32)
    res = None
    last_err = None
    for attempt in range(3):
        try:
            res = run_bass_kernel_spmd(nc, in_maps,
                                       core_ids=list(range(NCORES)),
                                       **run_kwargs)
            break
        except Exception as e:  # transient NRT_EXEC_UNIT_UNRECOVERABLE etc.
            last_err = e
    if res is None:
        raise last_err
    outs = [res.results[c]["outT"] for c in range(NCORES)]
    out = unshard(outs, bias_vec)
    return out


# revision 21
# speedup vs baseline: 1.1866x; 1.0203x over previous
"""Causal self-attention (B=2, T=4096, C=768, H=12) on 8 TRN2 NeuronCores.

Sharding: core c -> batch c//4, heads 3*(c%4) .. 3*(c%4)+2.  Each core is
fully independent (no collectives): it computes qkv for its 3 heads from
x[b], runs causal flash attention, and produces the partial output
projection outT = (Y_heads @ W_proj[rows]).T of shape [C, T].  The host
sums the 4 per-batch partials, transposes, and adds the folded bias
(W_proj^T b_v + b_proj).

Per-core structure (all matmuls float32r, 1 cyc/row at N>=256):
  - qkT kept as 4 m-tiles A=[q0|q1], B=[k0|k1], C=[q2|k2], C'=[k2|q2]
    (host-packed weight layout) so every S^T matmul pair issues from PE
    row-groups {0,1} and {2,3} concurrently with NO duplicate copies:
    heads 0/1 pair across the partition halves of A/B; head 2 uses C/C'.
  - v in natural [T, 64] orientation with a ones column (softmax
    denominator rides in the PV matmul); the ones are injected by a K=1
    matmul against a constant pattern row, not DVE writes.
  - exp on ACT with no max subtraction (logits are bounded); diagonal
    k-tiles are column-compacted (only q >= k block computed) and masked
    with small [128,128/256] triangle multiplies on DVE.
  - softmax division: PSUM yt -> SBUF copy (frees the PSUM bank for the
    next group immediately), reciprocal_approx_fast, K=1 broadcast
    matmul, and one fused scalar_tensor_tensor multiply.
  - All side work (next group's QKV, previous group's output projection,
    epilogue broadcast/divide) flows through a deferred-thunk queue that
    is pumped between attention units, keeping the PE stream dense so
    the HAM clock gate stays at 8/8.
"""

import os
import sys

import numpy as np

for _p in ("/opt/trn_rl_repo", "/root/.axon_site/_ro/trn_rl_repo"):
    if os.path.isdir(_p) and _p not in sys.path:
        sys.path.insert(0, _p)

from collections import deque
from contextlib import ExitStack

import concourse.bacc as bacc
import concourse.bass as bass
import concourse.mybir as mybir
import concourse.tile as tile
from concourse.bass_utils import run_bass_kernel_spmd

F32 = mybir.dt.float32
F32R = mybir.dt.float32r
BF16 = mybir.dt.bfloat16
EXP = mybir.ActivationFunctionType.Exp
IS_GE = mybir.AluOpType.is_ge
MUL = mybir.AluOpType.mult

ECHO = int(os.environ.get("K_ECHO", "0"))

B, T_FULL, C = 2, 4096, 768
H, DH = 12, 64
HPC = 3                      # heads per core
NCORES = 8
P = 128
QG = 512                     # query-group span
KT = 128                     # key tile
NQK = 4 * P                  # 512 rows of qkT (A, B, C, C')
NVP = 256                    # padded v width: v0|1|v1|1|v2|1|zeros
SCALE = 1.0 / np.sqrt(DH)


def r32(ap):
    return ap.bitcast(F32R)


def build_nc(t=T_FULL):
    ng = t // QG             # query groups
    nc = bacc.Bacc(None, target_bir_lowering=False)
    xT = nc.declare_dram_parameter("xT", [P, 6 * t], BF16, isOutput=False)
    wqk = nc.declare_dram_parameter("wqk", [P, 6 * NQK], BF16, isOutput=False)
    bqk = nc.declare_dram_parameter("bqk", [P, 4], F32, isOutput=False)
    wvp = nc.declare_dram_parameter("wvp", [P, 6 * NVP], BF16, isOutput=False)
    wp = nc.declare_dram_parameter("wp", [HPC * DH, C], BF16, isOutput=False)
    outT = nc.declare_dram_parameter("outT", [C, t], F32, isOutput=True)

    with tile.TileContext(nc) as tc, ExitStack() as ctx:
        const = ctx.enter_context(tc.tile_pool(name="const", bufs=1))
        qkp = ctx.enter_context(tc.tile_pool(name="qk", bufs=1))
        vp = ctx.enter_context(tc.tile_pool(name="vn", bufs=1))
        xpool = ctx.enter_context(tc.tile_pool(name="xin", bufs=2))
        ppool = ctx.enter_context(tc.tile_pool(name="pp", bufs=4))
        ytsb = ctx.enter_context(tc.tile_pool(name="ytsb", bufs=2))
        ysbp = ctx.enter_context(tc.tile_pool(name="ysb", bufs=3))
        rrp = ctx.enter_context(tc.tile_pool(name="rr", bufs=3))
        osb = ctx.enter_context(tc.tile_pool(name="osb", bufs=2))
        spsum = ctx.enter_context(tc.tile_pool(name="sps", bufs=2, space="PSUM"))
        ytps = ctx.enter_context(tc.tile_pool(name="ytps", bufs=2, space="PSUM"))
        aux = ctx.enter_context(tc.tile_pool(name="aux", bufs=2, space="PSUM"))

        # ---- constants (packed single-DMA loads) -----------------------
        wqk_all = const.tile([P, 6 * NQK], BF16, tag="wqka", name="wqka")
        nc.sync.dma_start(wqk_all[:], wqk[:, :])
        wqk_sb = [wqk_all[:, k * NQK:(k + 1) * NQK] for k in range(6)]
        wvp_all = const.tile([P, 6 * NVP], BF16, tag="wvpa", name="wvpa")
        nc.sync.dma_start(wvp_all[:], wvp[:, :])
        wvp_sb = [wvp_all[:, k * NVP:(k + 1) * NVP] for k in range(6)]
        bq_all = const.tile([P, 4], F32, tag="bqa", name="bqa")
        nc.sync.dma_start(bq_all[:], bqk[:, :])
        b_sb = [bq_all[:, m:m + 1] for m in range(4)]
        wp0 = const.tile([P, C], BF16, tag="wp0", name="wp0")
        wp1 = const.tile([DH, C], BF16, tag="wp1", name="wp1")
        nc.sync.dma_start(wp0[:], wp[0:P, :])
        nc.sync.dma_start(wp1[:], wp[P:P + DH, :])

        ones1 = const.tile([1, DH], BF16, tag="ones1", name="ones1")
        ones1t = const.tile([1, P], BF16, tag="ones1t", name="ones1t")
        vpat = const.tile([1, NVP], BF16, tag="vpat", name="vpat")
        tri1 = const.tile([P, KT], BF16, tag="tri1", name="tri1")
        tri2 = const.tile([P, 2 * KT], BF16, tag="tri2", name="tri2")
        with tc.tile_pool(name="scratch", bufs=1) as scratch:
            onesRF = scratch.tile([1, P], F32, tag="onesRF", name="onesRF")
            nc.vector.memset(onesRF[:], 1.0)
            nc.vector.tensor_copy(ones1[:], onesRF[:, 0:DH])
            nc.vector.tensor_copy(ones1t[:], onesRF[:])
            vpF = scratch.tile([1, NVP], F32, tag="vpF", name="vpF")
            nc.vector.memset(vpF[:], 0.0)
            for h in range(HPC):
                nc.vector.memset(vpF[:, 65 * h + DH:65 * h + DH + 1], 1.0)
            nc.vector.tensor_copy(vpat[:], vpF[:])
            # tri1[k, q] = 1 iff q >= k ; tri2[k, q] = 1 iff q >= k + 128
            trF = scratch.tile([P, 2 * KT], F32, tag="trF", name="trF")
            nc.gpsimd.memset(trF[:], 1.0)
            nc.gpsimd.affine_select(
                out=trF[:, 0:KT], in_=trF[:, 0:KT], compare_op=IS_GE,
                fill=0.0, base=0, pattern=[[1, KT]], channel_multiplier=-1,
            )
            nc.vector.tensor_copy(tri1[:], trF[:, 0:KT])
            nc.gpsimd.memset(trF[:], 1.0)
            nc.gpsimd.affine_select(
                out=trF[:], in_=trF[:], compare_op=IS_GE,
                fill=0.0, base=-KT, pattern=[[1, 2 * KT]], channel_multiplier=-1,
            )
            nc.vector.tensor_copy(tri2[:], trF[:])

        # ---- persistent qkT / v storage --------------------------------
        # A=[q0|q1] B=[k0|k1] C=[q2|k2] C'=[k2|q2]
        qkt = [qkp.tile([P, t], BF16, tag=f"qkt{i}", name=f"qkt{i}")
               for i in range(4)]
        A, Bt, Ct, Cp = qkt
        vnat = [vp.tile([P, NVP], BF16, tag=f"vn{j}", name=f"vn{j}")
                for j in range(t // P)]

        def v1ap(h, j):
            return vnat[j][:, 65 * h:65 * h + DH + 1]

        # ---- deferred PE-side work queue -------------------------------
        pe_q = deque()

        def pump(n=1):
            for _ in range(n):
                if not pe_q:
                    return
                pe_q.popleft()()

        # ---- per-group QKV emission ------------------------------------
        xtiles = {}

        def emit_qkv_unit(g, u):
            gs = slice(g * QG, (g + 1) * QG)
            if u == 0:
                xa = xpool.tile([P, 6 * QG], BF16, tag="xa", name="xa")
                src3 = xT[:, :].rearrange("p (k t) -> p k t", k=6)[:, :,
                                                                  gs]
                dst3 = xa[:].rearrange("p (k q) -> p k q", k=6)
                nc.sync.dma_start(dst3, src3)
                xtiles[g] = xa
                return
            xa = xtiles[g]

            def xk(k):
                return xa[:, k * QG:(k + 1) * QG]
            if u <= 4:
                m = u - 1           # qkT m-tile (A, B, C, C')
                ps = aux.tile([P, QG], F32, tag="aux", name="qkps")
                for k in range(6):
                    nc.tensor.matmul(ps[:], wqk_sb[k][:, m * P:(m + 1) * P],
                                     xk(k), start=(k == 0), stop=(k == 5))
                nc.vector.tensor_scalar_add(qkt[m][:, gs], ps[:], b_sb[m])
            else:
                ti = u - 5          # v t-tile within the group (0..3)
                j = 4 * g + ti
                ps = aux.tile([P, NVP], F32, tag="aux", name="vnps")
                for k in range(6):
                    nc.tensor.matmul(ps[:], xk(k)[:, ti * P:(ti + 1) * P],
                                     wvp_sb[k], start=(k == 0), stop=False)
                nc.tensor.matmul(ps[:], ones1t[:], vpat[:],
                                 start=False, stop=True)
                nc.vector.tensor_copy(vnat[j][:], ps[:])
                if u == 8:
                    xtiles.pop(g)   # release python ref (slots reused by tag)

        N_UNITS = 9  # 1 dma + 4 qk + 4 v

        # ---- attention unit (2 S^T blocks -> exp -> 2 PV) --------------
        def emit_unit(ytA, ytB, lhsA, lhsB, rhsA, rhsB, jA, jB, hA, hB,
                      w, qoff, rA, rB, startA, stopA, startB, stopB):
            """One unit: two S^T matmuls of width w at s2 cols 0 / 512,
            one exp over both blocks, triangle masks if diagonal, two PV
            matmuls accumulating into ytA/ytB cols [qoff:512].  rA/rB:
            None = no mask, >=0 = tri1 at block start, -1 = tri2 (block
            carries an extra 128 fully-masked columns; phase-B 2nd tile).
            """
            s2 = spsum.tile([P, 2 * QG], F32, tag="s", name="s")
            nc.tensor.matmul(s2[:, 0:w], lhsA, rhsA, start=True, stop=True)
            nc.tensor.matmul(s2[:, QG:QG + w], lhsB, rhsB,
                             start=True, stop=True)
            # Idempotent ballast: re-issue the S pair (same output, full
            # rewrite).  Keeps the PE array dense when no deferred work is
            # available, so the HAM clock gate stays at 8/8; the rewrite
            # produces identical values, and exp simply waits for the last.
            for _ in range(ECHO if not pe_q else 0):
                nc.tensor.matmul(s2[:, 0:w], lhsA, rhsA,
                                 start=True, stop=True)
                nc.tensor.matmul(s2[:, QG:QG + w], lhsB, rhsB,
                                 start=True, stop=True)
            p2 = ppool.tile([P, 2 * QG], BF16, tag="p", name="p")
            if w == QG:
                nc.scalar.activation(p2[:, 0:2 * QG], s2[:, 0:2 * QG], EXP,
                                     scale=float(SCALE))
            elif os.environ.get("K_NO_REARRANGE"):
                nc.scalar.activation(p2[:, 0:w], s2[:, 0:w], EXP,
                                     scale=float(SCALE))
                nc.scalar.activation(p2[:, QG:QG + w], s2[:, QG:QG + w], EXP,
                                     scale=float(SCALE))
            else:
                s3 = s2[:].rearrange("p (a b) -> p a b", a=2)[:, :, 0:w]
                p3 = p2[:].rearrange("p (a b) -> p a b", a=2)[:, :, 0:w]
                nc.scalar.activation(p3, s3, EXP, scale=float(SCALE))
            if rA is not None:
                nc.vector.tensor_mul(p2[:, 0:KT], p2[:, 0:KT], tri1[:])
            if rB is not None:
                if rB >= 0:
                    nc.vector.tensor_mul(p2[:, QG:QG + KT], p2[:, QG:QG + KT],
                                         tri1[:])
                else:
                    nc.vector.tensor_mul(p2[:, QG:QG + 2 * KT],
                                         p2[:, QG:QG + 2 * KT], tri2[:])
            nc.tensor.matmul(ytA[:, qoff:QG], v1ap(hA, jA), p2[:, 0:w],
                             start=startA, stop=stopA)
            nc.tensor.matmul(ytB[:, qoff:QG], v1ap(hB, jB),
                             p2[:, QG:QG + w],
                             start=startB, stop=stopB)

        # ---- epilogue --------------------------------------------------
        def emit_epilogue(g, h, yt_ps, dest):
            ysb = ysbp.tile([DH + 1, QG], F32, tag="ysb", name="ysb")
            nc.vector.tensor_copy(ysb[:], yt_ps[:])
            # reciprocal_approx_fast corrupts when its input AP sits at a
            # nonzero base partition: bounce the denominator to partition 0.
            d_f = rrp.tile([1, QG], F32, tag="df", name="df")
            nc.vector.tensor_copy(d_f[:], ysb[DH:DH + 1, :])
            r_f = rrp.tile([1, QG], F32, tag="rf", name="rf")
            nc.vector.reciprocal_approx_fast(r_f[:], d_f[:])
            r_r = rrp.tile([1, QG], BF16, tag="rr", name="rr")
            nc.vector.tensor_copy(r_r[:], r_f[:])

            def finish():
                R_t = aux.tile([DH, QG], F32, tag="aux", name="Rb")
                nc.tensor.matmul(R_t[:], ones1[:], r_r[:],
                                 start=True, stop=True)
                nc.vector.scalar_tensor_tensor(
                    out=dest, in0=R_t[:], scalar=1.0, in1=ysb[0:DH, :],
                    op0=MUL, op1=MUL)
            pe_q.append(finish)

        # ---- output projection (deferred) ------------------------------
        def push_proj(g, y0, y1):
            gs = slice(g * QG, (g + 1) * QG)

            def mk(cm):
                def run():
                    op = aux.tile([P, QG], F32, tag="aux", name="oo")
                    nc.tensor.matmul(op[:], wp0[:, cm * P:(cm + 1) * P],
                                     y0[:], start=True, stop=False)
                    nc.tensor.matmul(op[:], wp1[:, cm * P:(cm + 1) * P],
                                     y1[:], start=False, stop=True)
                    ob = osb.tile([P, QG], F32, tag="ob", name="ob")
                    nc.vector.tensor_copy(ob[:], op[:])
                    nc.sync.dma_start(outT[cm * P:(cm + 1) * P, gs], ob[:])
                return run
            for cm in range(6):
                pe_q.append(mk(cm))

        # ---- prologue: group 0's QKV -----------------------------------
        for u in range(N_UNITS):
            emit_qkv_unit(0, u)

        # ---- main loop --------------------------------------------------
        for g in range(ng):
            nkt = 4 * (g + 1)    # k-tiles this group
            gq0 = g * QG
            if g + 1 < ng:
                for u in range(N_UNITS):
                    uu = u
                    pe_q.append(lambda gg=g + 1, uu=uu: emit_qkv_unit(gg, uu))

            n_slots = 6 * (g + 1) + 3
            slot = 0
            popped = 0

            def pump(n=1):
                nonlocal popped
                for _ in range(n):
                    if not pe_q:
                        return
                    pe_q.popleft()()
                    popped += 1

            def pump_slot():
                # uniform spread of deferred work across the whole group so
                # fills remain available to cover late phase-boundary stalls
                nonlocal slot
                slot += 1
                if slot >= n_slots:
                    pump(len(pe_q))
                else:
                    total = popped + len(pe_q)
                    want = total * slot // n_slots
                    if want > popped:
                        pump(want - popped)

            yt01 = [ytps.tile([DH + 1, QG], F32, tag="yt", name="yt0p"),
                    ytps.tile([DH + 1, QG], F32, tag="yt", name="yt1p")]
            y0 = ytsb.tile([P, QG], BF16, tag="yt0", name="yt0")
            y1 = ytsb.tile([DH, QG], BF16, tag="yt1", name="yt1")

            # ---- phase A: heads 0 & 1, one k-tile per unit -------------
            for j in range(nkt):
                r = j - 4 * g
                w = QG if r < 0 else QG - KT * r
                qoff = QG - w
                tc0 = j * KT
                qs = slice(gq0 + qoff, gq0 + QG)
                emit_unit(
                    yt01[0], yt01[1],
                    Bt[0:DH, tc0:tc0 + KT], Bt[DH:P, tc0:tc0 + KT],
                    A[0:DH, qs], A[DH:P, qs],
                    j, j, 0, 1, w, qoff,
                    (r if r >= 0 else None), (r if r >= 0 else None),
                    startA=(j == 0), stopA=(j == nkt - 1),
                    startB=(j == 0), stopB=(j == nkt - 1))
                pump_slot()
            emit_epilogue(g, 0, yt01[0], y0[0:DH, :])
            pump_slot()
            emit_epilogue(g, 1, yt01[1], y0[DH:P, :])
            pump_slot()

            # ---- phase B: head 2, two k-tiles per unit -----------------
            yt2 = ytps.tile([DH + 1, QG], F32, tag="yt", name="yt2p")
            npr = 2 * (g + 1)
            for pr in range(npr):
                j0, j1 = 2 * pr, 2 * pr + 1
                r0 = j0 - 4 * g
                w0 = QG if r0 < 0 else QG - KT * r0
                qoff = QG - w0
                qs = slice(gq0 + qoff, gq0 + QG)
                emit_unit(
                    yt2, yt2,
                    Cp[0:DH, j0 * KT:(j0 + 1) * KT],
                    Ct[DH:P, j1 * KT:(j1 + 1) * KT],
                    Ct[0:DH, qs], Cp[DH:P, qs],
                    j0, j1, 2, 2, w0, qoff,
                    (r0 if r0 >= 0 else None),
                    (-1 if r0 >= 0 else None),   # -1 -> tri2 on block B
                    startA=(pr == 0), stopA=False,
                    startB=False, stopB=(pr == npr - 1))
                pump_slot()
            emit_epilogue(g, 2, yt2, y1[:])
            pump_slot()

            push_proj(g, y0, y1)

        while pe_q:
            pump()
    nc.compile()
    return nc


_NC_CACHE = {}


def get_nc(t=T_FULL):
    if t not in _NC_CACHE:
        _NC_CACHE[t] = build_nc(t)
    return _NC_CACHE[t]


def make_in_maps(x, W_attn, b_attn, W_proj):
    import ml_dtypes
    bf16 = ml_dtypes.bfloat16
    x = np.ascontiguousarray(np.asarray(x, np.float32))
    W_attn = np.asarray(W_attn, np.float32)
    b_attn = np.asarray(b_attn, np.float32)
    W_proj = np.asarray(W_proj, np.float32)

    def qcol(h):
        return W_attn[:, h * DH:(h + 1) * DH]

    def kcol(h):
        return W_attn[:, C + h * DH:C + (h + 1) * DH]

    def qb(h):
        return b_attn[h * DH:(h + 1) * DH]

    def kb(h):
        return b_attn[C + h * DH:C + (h + 1) * DH]

    in_maps = []
    for c in range(NCORES):
        b = c // 4
        hs = [3 * (c % 4) + i for i in range(HPC)]
        # A=[q0|q1] B=[k0|k1] C=[q2|k2] C'=[k2|q2]
        cols = [qcol(hs[0]), qcol(hs[1]), kcol(hs[0]), kcol(hs[1]),
                qcol(hs[2]), kcol(hs[2]), kcol(hs[2]), qcol(hs[2])]
        wqk = np.ascontiguousarray(np.concatenate(cols, axis=1))
        bqk = np.concatenate(
            [qb(hs[0]), qb(hs[1]), kb(hs[0]), kb(hs[1]),
             qb(hs[2]), kb(hs[2]), kb(hs[2]), qb(hs[2])]
        ).reshape(NQK, 1)
        wvp = np.zeros((C, NVP), np.float32)
        for lh in range(HPC):
            wvp[:, 65 * lh:65 * lh + DH] = \
                W_attn[:, 2 * C + hs[lh] * DH:2 * C + (hs[lh] + 1) * DH]
        wp = np.ascontiguousarray(
            np.concatenate([W_proj[h * DH:(h + 1) * DH, :] for h in hs], axis=0)
        )
        xTc = x[b].T                      # [C, t]
        t = xTc.shape[1]
        xTp = np.concatenate([xTc[k * P:(k + 1) * P, :] for k in range(6)],
                             axis=1)      # [128, 6t]
        wqkp = np.concatenate([wqk[k * P:(k + 1) * P, :] for k in range(6)],
                              axis=1)     # [128, 6*NQK]
        wvpp = np.concatenate([wvp[k * P:(k + 1) * P, :] for k in range(6)],
                              axis=1)     # [128, 6*NVP]
        bqkp = bqk.reshape(4, P).T        # [128, 4]
        in_maps.append({
            "xT": np.ascontiguousarray(xTp).astype(bf16),
            "wqk": np.ascontiguousarray(wqkp).astype(bf16),
            "bqk": np.ascontiguousarray(bqkp, dtype=np.float32),
            "wvp": np.ascontiguousarray(wvpp).astype(bf16),
            "wp": wp.astype(bf16),
        })
    return in_maps


def unshard(per_core_outT, bias_vec):
    t = per_core_outT[0].shape[1]
    out = np.zeros((B, t, C), np.float32)
    for c in range(NCORES):
        out[c // 4] += per_core_outT[c].T
    out += np.asarray(bias_vec, np.float32)[None, None, :]
    return out


def kernel(x, W_attn, b_attn, W_proj, b_proj, **run_kwargs):
    nc = get_nc(T_FULL)
    in_maps = make_in_maps(x, W_attn, b_attn, W_proj)
    # v-bias and b_proj fold into one per-channel constant:
    # out = sum_h Wp_h^T (attn_h) + (b_v @ W_proj + b_proj)
    b_attn = np.asarray(b_attn, np.float32)
    bias_vec = b_attn[2 * C:] @ np.asarray(W_proj, np.float32) \
        + np.asarray(b_proj, np.float32)
    res = None
    last_err = None
    for attempt in range(3):
        try:
            res = run_bass_kernel_spmd(nc, in_maps,
                                       core_ids=list(range(NCORES)),
                                       **run_kwargs)
            break
        except Exception as e:  # transient NRT_EXEC_UNIT_UNRECOVERABLE etc.
            last_err = e
    if res is None:
        raise last_err
    outs = [res.results[c]["outT"] for c in range(NCORES)]
    out = unshard(outs, bias_vec)
    return out


# revision 22
# speedup vs baseline: 1.1874x; 1.0007x over previous
"""Causal self-attention (B=2, T=4096, C=768, H=12) on 8 TRN2 NeuronCores.

Sharding: core c -> batch c//4, heads 3*(c%4) .. 3*(c%4)+2.  Each core is
fully independent (no collectives): it computes qkv for its 3 heads from
x[b], runs causal flash attention, and produces the partial output
projection outT = (Y_heads @ W_proj[rows]).T of shape [C, T].  The host
sums the 4 per-batch partials, transposes, and adds the folded bias
(W_proj^T b_v + b_proj).

Per-core structure (all matmuls float32r, 1 cyc/row at N>=256):
  - qkT kept as 4 m-tiles A=[q0|q1], B=[k0|k1], C=[q2|k2], C'=[k2|q2]
    (host-packed weight layout) so every S^T matmul pair issues from PE
    row-groups {0,1} and {2,3} concurrently with NO duplicate copies:
    heads 0/1 pair across the partition halves of A/B; head 2 uses C/C'.
  - v in natural [T, 64] orientation with a ones column (softmax
    denominator rides in the PV matmul); the ones are injected by a K=1
    matmul against a constant pattern row, not DVE writes.
  - exp on ACT with no max subtraction (logits are bounded); diagonal
    k-tiles are column-compacted (only q >= k block computed) and masked
    with small [128,128/256] triangle multiplies on DVE.
  - softmax division: PSUM yt -> SBUF copy (frees the PSUM bank for the
    next group immediately), reciprocal_approx_fast, K=1 broadcast
    matmul, and one fused scalar_tensor_tensor multiply.
  - All side work (next group's QKV, previous group's output projection,
    epilogue broadcast/divide) flows through a deferred-thunk queue that
    is pumped between attention units, keeping the PE stream dense so
    the HAM clock gate stays at 8/8.
"""

import os
import sys

import numpy as np

for _p in ("/opt/trn_rl_repo", "/root/.axon_site/_ro/trn_rl_repo"):
    if os.path.isdir(_p) and _p not in sys.path:
        sys.path.insert(0, _p)

from collections import deque
from contextlib import ExitStack

import concourse.bacc as bacc
import concourse.bass as bass
import concourse.mybir as mybir
import concourse.tile as tile
from concourse.bass_utils import run_bass_kernel_spmd

F32 = mybir.dt.float32
F32R = mybir.dt.float32r
BF16 = mybir.dt.bfloat16
EXP = mybir.ActivationFunctionType.Exp
IS_GE = mybir.AluOpType.is_ge
MUL = mybir.AluOpType.mult

ECHO = int(os.environ.get("K_ECHO", "0"))

B, T_FULL, C = 2, 4096, 768
H, DH = 12, 64
HPC = 3                      # heads per core
NCORES = 8
P = 128
QG = 512                     # query-group span
KT = 128                     # key tile
NQK = 4 * P                  # 512 rows of qkT (A, B, C, C')
NVP = 256                    # padded v width: v0|1|v1|1|v2|1|zeros
SCALE = 1.0 / np.sqrt(DH)


def r32(ap):
    return ap.bitcast(F32R)


def build_nc(t=T_FULL):
    ng = t // QG             # query groups
    nc = bacc.Bacc(None, target_bir_lowering=False)
    xT = nc.declare_dram_parameter("xT", [P, 6 * t], BF16, isOutput=False)
    wqk = nc.declare_dram_parameter("wqk", [P, 6 * NQK], BF16, isOutput=False)
    bqk = nc.declare_dram_parameter("bqk", [P, 4], F32, isOutput=False)
    wvp = nc.declare_dram_parameter("wvp", [P, 6 * NVP], BF16, isOutput=False)
    wp = nc.declare_dram_parameter("wp", [HPC * DH, C], BF16, isOutput=False)
    outT = nc.declare_dram_parameter("outT", [C, t], F32, isOutput=True)

    with tile.TileContext(nc) as tc, ExitStack() as ctx:
        const = ctx.enter_context(tc.tile_pool(name="const", bufs=1))
        qkp = ctx.enter_context(tc.tile_pool(name="qk", bufs=1))
        vp = ctx.enter_context(tc.tile_pool(name="vn", bufs=1))
        xpool = ctx.enter_context(tc.tile_pool(name="xin", bufs=2))
        ppool = ctx.enter_context(tc.tile_pool(name="pp", bufs=4))
        ytsb = ctx.enter_context(tc.tile_pool(name="ytsb", bufs=2))
        ysbp = ctx.enter_context(tc.tile_pool(name="ysb", bufs=4))
        rrp = ctx.enter_context(tc.tile_pool(name="rr", bufs=4))
        osb = ctx.enter_context(tc.tile_pool(name="osb", bufs=4))
        spsum = ctx.enter_context(tc.tile_pool(name="sps", bufs=2, space="PSUM"))
        ytps = ctx.enter_context(tc.tile_pool(name="ytps", bufs=2, space="PSUM"))
        aux = ctx.enter_context(tc.tile_pool(name="aux", bufs=2, space="PSUM"))

        # ---- constants (packed single-DMA loads) -----------------------
        wqk_all = const.tile([P, 6 * NQK], BF16, tag="wqka", name="wqka")
        nc.sync.dma_start(wqk_all[:], wqk[:, :])
        wqk_sb = [wqk_all[:, k * NQK:(k + 1) * NQK] for k in range(6)]
        wvp_all = const.tile([P, 6 * NVP], BF16, tag="wvpa", name="wvpa")
        nc.sync.dma_start(wvp_all[:], wvp[:, :])
        wvp_sb = [wvp_all[:, k * NVP:(k + 1) * NVP] for k in range(6)]
        bq_all = const.tile([P, 4], F32, tag="bqa", name="bqa")
        nc.sync.dma_start(bq_all[:], bqk[:, :])
        b_sb = [bq_all[:, m:m + 1] for m in range(4)]
        wp0 = const.tile([P, C], BF16, tag="wp0", name="wp0")
        wp1 = const.tile([DH, C], BF16, tag="wp1", name="wp1")
        nc.sync.dma_start(wp0[:], wp[0:P, :])
        nc.sync.dma_start(wp1[:], wp[P:P + DH, :])

        ones1 = const.tile([1, DH], BF16, tag="ones1", name="ones1")
        ones1t = const.tile([1, P], BF16, tag="ones1t", name="ones1t")
        vpat = const.tile([1, NVP], BF16, tag="vpat", name="vpat")
        tri1 = const.tile([P, KT], BF16, tag="tri1", name="tri1")
        tri2 = const.tile([P, 2 * KT], BF16, tag="tri2", name="tri2")
        with tc.tile_pool(name="scratch", bufs=1) as scratch:
            onesRF = scratch.tile([1, P], F32, tag="onesRF", name="onesRF")
            nc.vector.memset(onesRF[:], 1.0)
            nc.vector.tensor_copy(ones1[:], onesRF[:, 0:DH])
            nc.vector.tensor_copy(ones1t[:], onesRF[:])
            vpF = scratch.tile([1, NVP], F32, tag="vpF", name="vpF")
            nc.vector.memset(vpF[:], 0.0)
            for h in range(HPC):
                nc.vector.memset(vpF[:, 65 * h + DH:65 * h + DH + 1], 1.0)
            nc.vector.tensor_copy(vpat[:], vpF[:])
            # tri1[k, q] = 1 iff q >= k ; tri2[k, q] = 1 iff q >= k + 128
            trF = scratch.tile([P, 2 * KT], F32, tag="trF", name="trF")
            nc.gpsimd.memset(trF[:], 1.0)
            nc.gpsimd.affine_select(
                out=trF[:, 0:KT], in_=trF[:, 0:KT], compare_op=IS_GE,
                fill=0.0, base=0, pattern=[[1, KT]], channel_multiplier=-1,
            )
            nc.vector.tensor_copy(tri1[:], trF[:, 0:KT])
            nc.gpsimd.memset(trF[:], 1.0)
            nc.gpsimd.affine_select(
                out=trF[:], in_=trF[:], compare_op=IS_GE,
                fill=0.0, base=-KT, pattern=[[1, 2 * KT]], channel_multiplier=-1,
            )
            nc.vector.tensor_copy(tri2[:], trF[:])

        # ---- persistent qkT / v storage --------------------------------
        # A=[q0|q1] B=[k0|k1] C=[q2|k2] C'=[k2|q2]
        qkt = [qkp.tile([P, t], BF16, tag=f"qkt{i}", name=f"qkt{i}")
               for i in range(4)]
        A, Bt, Ct, Cp = qkt
        vnat = [vp.tile([P, NVP], BF16, tag=f"vn{j}", name=f"vn{j}")
                for j in range(t // P)]

        def v1ap(h, j):
            return vnat[j][:, 65 * h:65 * h + DH + 1]

        # ---- deferred PE-side work queue -------------------------------
        pe_q = deque()

        def pump(n=1):
            for _ in range(n):
                if not pe_q:
                    return
                pe_q.popleft()()

        # ---- per-group QKV emission ------------------------------------
        xtiles = {}

        def emit_qkv_unit(g, u):
            gs = slice(g * QG, (g + 1) * QG)
            if u == 0:
                xa = xpool.tile([P, 6 * QG], BF16, tag="xa", name="xa")
                src3 = xT[:, :].rearrange("p (k t) -> p k t", k=6)[:, :,
                                                                  gs]
                dst3 = xa[:].rearrange("p (k q) -> p k q", k=6)
                nc.sync.dma_start(dst3, src3)
                xtiles[g] = xa
                return
            xa = xtiles[g]

            def xk(k):
                return xa[:, k * QG:(k + 1) * QG]
            if u <= 4:
                m = u - 1           # qkT m-tile (A, B, C, C')
                ps = aux.tile([P, QG], F32, tag="aux", name="qkps")
                for k in range(6):
                    nc.tensor.matmul(ps[:], wqk_sb[k][:, m * P:(m + 1) * P],
                                     xk(k), start=(k == 0), stop=(k == 5))
                nc.vector.tensor_scalar_add(qkt[m][:, gs], ps[:], b_sb[m])
            else:
                ti = u - 5          # v t-tile within the group (0..3)
                j = 4 * g + ti
                ps = aux.tile([P, NVP], F32, tag="aux", name="vnps")
                for k in range(6):
                    nc.tensor.matmul(ps[:], xk(k)[:, ti * P:(ti + 1) * P],
                                     wvp_sb[k], start=(k == 0), stop=False)
                nc.tensor.matmul(ps[:], ones1t[:], vpat[:],
                                 start=False, stop=True)
                nc.vector.tensor_copy(vnat[j][:], ps[:])
                if u == 8:
                    xtiles.pop(g)   # release python ref (slots reused by tag)

        N_UNITS = 9  # 1 dma + 4 qk + 4 v

        # ---- attention unit (2 S^T blocks -> exp -> 2 PV) --------------
        def emit_unit(ytA, ytB, lhsA, lhsB, rhsA, rhsB, jA, jB, hA, hB,
                      w, qoff, rA, rB, startA, stopA, startB, stopB):
            """One unit: two S^T matmuls of width w at s2 cols 0 / 512,
            one exp over both blocks, triangle masks if diagonal, two PV
            matmuls accumulating into ytA/ytB cols [qoff:512].  rA/rB:
            None = no mask, >=0 = tri1 at block start, -1 = tri2 (block
            carries an extra 128 fully-masked columns; phase-B 2nd tile).
            """
            s2 = spsum.tile([P, 2 * QG], F32, tag="s", name="s")
            nc.tensor.matmul(s2[:, 0:w], lhsA, rhsA, start=True, stop=True)
            nc.tensor.matmul(s2[:, QG:QG + w], lhsB, rhsB,
                             start=True, stop=True)
            # Idempotent ballast: re-issue the S pair (same output, full
            # rewrite).  Keeps the PE array dense when no deferred work is
            # available, so the HAM clock gate stays at 8/8; the rewrite
            # produces identical values, and exp simply waits for the last.
            for _ in range(ECHO if not pe_q else 0):
                nc.tensor.matmul(s2[:, 0:w], lhsA, rhsA,
                                 start=True, stop=True)
                nc.tensor.matmul(s2[:, QG:QG + w], lhsB, rhsB,
                                 start=True, stop=True)
            p2 = ppool.tile([P, 2 * QG], BF16, tag="p", name="p")
            if w == QG:
                nc.scalar.activation(p2[:, 0:2 * QG], s2[:, 0:2 * QG], EXP,
                                     scale=float(SCALE))
            elif os.environ.get("K_NO_REARRANGE"):
                nc.scalar.activation(p2[:, 0:w], s2[:, 0:w], EXP,
                                     scale=float(SCALE))
                nc.scalar.activation(p2[:, QG:QG + w], s2[:, QG:QG + w], EXP,
                                     scale=float(SCALE))
            else:
                s3 = s2[:].rearrange("p (a b) -> p a b", a=2)[:, :, 0:w]
                p3 = p2[:].rearrange("p (a b) -> p a b", a=2)[:, :, 0:w]
                nc.scalar.activation(p3, s3, EXP, scale=float(SCALE))
            if rA is not None:
                nc.vector.tensor_mul(p2[:, 0:KT], p2[:, 0:KT], tri1[:])
            if rB is not None:
                if rB >= 0:
                    nc.vector.tensor_mul(p2[:, QG:QG + KT], p2[:, QG:QG + KT],
                                         tri1[:])
                else:
                    nc.vector.tensor_mul(p2[:, QG:QG + 2 * KT],
                                         p2[:, QG:QG + 2 * KT], tri2[:])
            nc.tensor.matmul(ytA[:, qoff:QG], v1ap(hA, jA), p2[:, 0:w],
                             start=startA, stop=stopA)
            nc.tensor.matmul(ytB[:, qoff:QG], v1ap(hB, jB),
                             p2[:, QG:QG + w],
                             start=startB, stop=stopB)

        # ---- epilogue --------------------------------------------------
        def emit_epilogue(g, h, yt_ps, dest):
            ysb = ysbp.tile([DH + 1, QG], F32, tag="ysb", name="ysb")
            nc.vector.tensor_copy(ysb[:], yt_ps[:])
            # reciprocal_approx_fast corrupts when its input AP sits at a
            # nonzero base partition: bounce the denominator to partition 0.
            d_f = rrp.tile([1, QG], F32, tag="df", name="df")
            nc.vector.tensor_copy(d_f[:], ysb[DH:DH + 1, :])
            r_f = rrp.tile([1, QG], F32, tag="rf", name="rf")
            nc.vector.reciprocal_approx_fast(r_f[:], d_f[:])
            r_r = rrp.tile([1, QG], BF16, tag="rr", name="rr")
            nc.vector.tensor_copy(r_r[:], r_f[:])

            def finish():
                R_t = aux.tile([DH, QG], F32, tag="aux", name="Rb")
                nc.tensor.matmul(R_t[:], ones1[:], r_r[:],
                                 start=True, stop=True)
                nc.vector.scalar_tensor_tensor(
                    out=dest, in0=R_t[:], scalar=1.0, in1=ysb[0:DH, :],
                    op0=MUL, op1=MUL)
            pe_q.append(finish)

        # ---- output projection (deferred) ------------------------------
        def push_proj(g, y0, y1):
            gs = slice(g * QG, (g + 1) * QG)

            def mk(cm):
                def run():
                    op = aux.tile([P, QG], F32, tag="aux", name="oo")
                    nc.tensor.matmul(op[:], wp0[:, cm * P:(cm + 1) * P],
                                     y0[:], start=True, stop=False)
                    nc.tensor.matmul(op[:], wp1[:, cm * P:(cm + 1) * P],
                                     y1[:], start=False, stop=True)
                    ob = osb.tile([P, QG], F32, tag="ob", name="ob")
                    nc.vector.tensor_copy(ob[:], op[:])
                    nc.sync.dma_start(outT[cm * P:(cm + 1) * P, gs], ob[:])
                return run
            for cm in range(6):
                pe_q.append(mk(cm))

        # ---- prologue: group 0's QKV -----------------------------------
        for u in range(N_UNITS):
            emit_qkv_unit(0, u)

        # ---- main loop --------------------------------------------------
        for g in range(ng):
            nkt = 4 * (g + 1)    # k-tiles this group
            gq0 = g * QG
            if g + 1 < ng:
                for u in range(N_UNITS):
                    uu = u
                    pe_q.append(lambda gg=g + 1, uu=uu: emit_qkv_unit(gg, uu))

            n_slots = 6 * (g + 1) + 3
            slot = 0
            popped = 0

            def pump(n=1):
                nonlocal popped
                for _ in range(n):
                    if not pe_q:
                        return
                    pe_q.popleft()()
                    popped += 1

            def pump_slot():
                # uniform spread of deferred work across the whole group so
                # fills remain available to cover late phase-boundary stalls
                nonlocal slot
                slot += 1
                if slot >= n_slots:
                    pump(len(pe_q))
                else:
                    total = popped + len(pe_q)
                    want = total * slot // n_slots
                    if want > popped:
                        pump(want - popped)

            yt01 = [ytps.tile([DH + 1, QG], F32, tag="yt", name="yt0p"),
                    ytps.tile([DH + 1, QG], F32, tag="yt", name="yt1p")]
            y0 = ytsb.tile([P, QG], BF16, tag="yt0", name="yt0")
            y1 = ytsb.tile([DH, QG], BF16, tag="yt1", name="yt1")

            # ---- phase A: heads 0 & 1, one k-tile per unit -------------
            for j in range(nkt):
                r = j - 4 * g
                w = QG if r < 0 else QG - KT * r
                qoff = QG - w
                tc0 = j * KT
                qs = slice(gq0 + qoff, gq0 + QG)
                emit_unit(
                    yt01[0], yt01[1],
                    Bt[0:DH, tc0:tc0 + KT], Bt[DH:P, tc0:tc0 + KT],
                    A[0:DH, qs], A[DH:P, qs],
                    j, j, 0, 1, w, qoff,
                    (r if r >= 0 else None), (r if r >= 0 else None),
                    startA=(j == 0), stopA=(j == nkt - 1),
                    startB=(j == 0), stopB=(j == nkt - 1))
                pump_slot()
            emit_epilogue(g, 0, yt01[0], y0[0:DH, :])
            pump_slot()
            emit_epilogue(g, 1, yt01[1], y0[DH:P, :])
            pump_slot()

            # ---- phase B: head 2, two k-tiles per unit -----------------
            yt2 = ytps.tile([DH + 1, QG], F32, tag="yt", name="yt2p")
            npr = 2 * (g + 1)
            for pr in range(npr):
                j0, j1 = 2 * pr, 2 * pr + 1
                r0 = j0 - 4 * g
                w0 = QG if r0 < 0 else QG - KT * r0
                qoff = QG - w0
                qs = slice(gq0 + qoff, gq0 + QG)
                emit_unit(
                    yt2, yt2,
                    Cp[0:DH, j0 * KT:(j0 + 1) * KT],
                    Ct[DH:P, j1 * KT:(j1 + 1) * KT],
                    Ct[0:DH, qs], Cp[DH:P, qs],
                    j0, j1, 2, 2, w0, qoff,
                    (r0 if r0 >= 0 else None),
                    (-1 if r0 >= 0 else None),   # -1 -> tri2 on block B
                    startA=(pr == 0), stopA=False,
                    startB=False, stopB=(pr == npr - 1))
                pump_slot()
            emit_epilogue(g, 2, yt2, y1[:])
            pump_slot()

            push_proj(g, y0, y1)

        while pe_q:
            pump()
    nc.compile()
    return nc


_NC_CACHE = {}


def get_nc(t=T_FULL):
    if t not in _NC_CACHE:
        _NC_CACHE[t] = build_nc(t)
    return _NC_CACHE[t]


def make_in_maps(x, W_attn, b_attn, W_proj):
    import ml_dtypes
    bf16 = ml_dtypes.bfloat16
    x = np.ascontiguousarray(np.asarray(x, np.float32))
    W_attn = np.asarray(W_attn, np.float32)
    b_attn = np.asarray(b_attn, np.float32)
    W_proj = np.asarray(W_proj, np.float32)

    def qcol(h):
        return W_attn[:, h * DH:(h + 1) * DH]

    def kcol(h):
        return W_attn[:, C + h * DH:C + (h + 1) * DH]

    def qb(h):
        return b_attn[h * DH:(h + 1) * DH]

    def kb(h):
        return b_attn[C + h * DH:C + (h + 1) * DH]

    in_maps = []
    for c in range(NCORES):
        b = c // 4
        hs = [3 * (c % 4) + i for i in range(HPC)]
        # A=[q0|q1] B=[k0|k1] C=[q2|k2] C'=[k2|q2]
        cols = [qcol(hs[0]), qcol(hs[1]), kcol(hs[0]), kcol(hs[1]),
                qcol(hs[2]), kcol(hs[2]), kcol(hs[2]), qcol(hs[2])]
        wqk = np.ascontiguousarray(np.concatenate(cols, axis=1))
        bqk = np.concatenate(
            [qb(hs[0]), qb(hs[1]), kb(hs[0]), kb(hs[1]),
             qb(hs[2]), kb(hs[2]), kb(hs[2]), qb(hs[2])]
        ).reshape(NQK, 1)
        wvp = np.zeros((C, NVP), np.float32)
        for lh in range(HPC):
            wvp[:, 65 * lh:65 * lh + DH] = \
                W_attn[:, 2 * C + hs[lh] * DH:2 * C + (hs[lh] + 1) * DH]
        wp = np.ascontiguousarray(
            np.concatenate([W_proj[h * DH:(h + 1) * DH, :] for h in hs], axis=0)
        )
        xTc = x[b].T                      # [C, t]
        t = xTc.shape[1]
        xTp = np.concatenate([xTc[k * P:(k + 1) * P, :] for k in range(6)],
                             axis=1)      # [128, 6t]
        wqkp = np.concatenate([wqk[k * P:(k + 1) * P, :] for k in range(6)],
                              axis=1)     # [128, 6*NQK]
        wvpp = np.concatenate([wvp[k * P:(k + 1) * P, :] for k in range(6)],
                              axis=1)     # [128, 6*NVP]
        bqkp = bqk.reshape(4, P).T        # [128, 4]
        in_maps.append({
            "xT": np.ascontiguousarray(xTp).astype(bf16),
            "wqk": np.ascontiguousarray(wqkp).astype(bf16),
            "bqk": np.ascontiguousarray(bqkp, dtype=np.float32),
            "wvp": np.ascontiguousarray(wvpp).astype(bf16),
            "wp": wp.astype(bf16),
        })
    return in_maps


def unshard(per_core_outT, bias_vec):
    t = per_core_outT[0].shape[1]
    out = np.zeros((B, t, C), np.float32)
    for c in range(NCORES):
        out[c // 4] += per_core_outT[c].T
    out += np.asarray(bias_vec, np.float32)[None, None, :]
    return out


def kernel(x, W_attn, b_attn, W_proj, b_proj, **run_kwargs):
    nc = get_nc(T_FULL)
    in_maps = make_in_maps(x, W_attn, b_attn, W_proj)
    # v-bias and b_proj fold into one per-channel constant:
    # out = sum_h Wp_h^T (attn_h) + (b_v @ W_proj + b_proj)
    b_attn = np.asarray(b_attn, np.float32)
    bias_vec = b_attn[2 * C:] @ np.asarray(W_proj, np.float32) \
        + np.asarray(b_proj, np.float32)
    res = None
    last_err = None
    for attempt in range(3):
        try:
            res = run_bass_kernel_spmd(nc, in_maps,
                                       core_ids=list(range(NCORES)),
                                       **run_kwargs)
            break
        except Exception as e:  # transient NRT_EXEC_UNIT_UNRECOVERABLE etc.
            last_err = e
    if res is None:
        raise last_err
    outs = [res.results[c]["outT"] for c in range(NCORES)]
    out = unshard(outs, bias_vec)
    return out
